# revision 7
# baseline (speedup 1.0000x reference)
"""MoE gate kernel for TRN2: logits = x @ w, top-8 over 64 experts, softmax.

Sharding: x [65536, 1024] split by token across 8 cores (8192 tokens each),
fed pre-transposed + pre-tiled. w [1024, 64] replicated.

Precision: x shipped as exact fp16 hi/lo pair (x == xh + xl + O(2^-22)),
w split into fp16 hi/lo on device (same DVE conversions as the proven
kernel, so wh/wl are bit-identical to it). The two w halves are packed
side by side into one [128, 128] stationary [wh | wl], so ONE moving
pass of xh computes xh@wh (PSUM partitions 0-63, "B") AND xh@wl
(partitions 64-127, "W") simultaneously; the xl pass adds xl@wh / xl@wl.
16 matmuls/chunk; B's partial order (xh@wh k0..7, xl@wh k0..7) is
bit-identical to the proven kernel, and W accumulates at magnitude
~2^-11 so it is error-free.

Top-8 robustness: a plain fl(B+W) comparison carries +-ulp(34) ~ 2e-6
noise, which mis-orders a sub-ULP logit pair in the eval set.  Instead
the logit is kept as an exact Dekker pair: s = fl(B+W), err = W-(s-B)
(TwoSum; s+err == B+W exactly), and the selection key is
r = fl(fl(s - m) + err) with m = per-token max(s).  s-m is exact for
all top-8 contenders (Sterbenz: s >= m/2), so r orders contenders with
error <= ulp(~10)/2 ~ 2.4e-7, an order of magnitude below every
remaining decision margin.  Softmax is shift-invariant, so feeding it
the r-values directly yields the same scores.

DMA: only sync/scalar/gpsimd can issue DMAs.  Measured under 3-way
contention the HBM arbiter grants ~126.9/125.8/105.6 GB/s respectively
(sum = the 358 GB/s per-core HBM cap), so the 64 x-pieces (512KB
halves; first/last two chunks quartered for ramp/finish balance) are
assigned by a static greedy earliest-finish schedule weighted by those
rates, so all three queues stream gaplessly and finish together.
gpsimd's stream carries ONLY x loads (plus the initial w load): output
DMAs were moved off it because their in-stream waits on the compute
chain were starving the SWDGE queue (~32us of gaps).  Scores/experts
accumulate in two resident SBUF tiles and flush as 2+2 batched DMAs on
sync: chunks 0..13 during the drain window, the last two chunks as a
~0.5us serial tail.  7 chunks of x buffer (xpool bufs=14) keep DMA
issue ahead of compute.
"""

import sys

sys.path.insert(0, "/opt/trn_rl_repo")

from contextlib import ExitStack

import numpy as np

import concourse.bacc as bacc
import concourse.mybir as mybir
import concourse.tile as tile
from concourse import masks
from concourse.bass_utils import run_bass_kernel_spmd

N_CORES = 8
TOKENS = 65536
D = 1024
E = 64
TOPK = 8
TOK_PER_CORE = TOKENS // N_CORES
CHUNK = 512  # tokens per chunk (PSUM bank = 512 f32)
SUBS = CHUNK // 128
KCH = D // 128  # contraction chunks
N_CHUNK = TOK_PER_CORE // CHUNK
HKCH = KCH // 2  # k-chunks per DMA half
DELTA = 6  # software pipeline depth (chunks)
PIPE = 7  # chunks of x resident in SBUF

# throttle-averaged per-queue rates (GB/s): the DVS governor duty-cycles a
# 50% util limit (~3.4us quanta) once the kernel runs hot; during k=4
# windows every queue is pinned at 104.9, during k=8 the HWDGE queues reach
# 127-206 while SWDGE stays ~105.  Averages below; lag = queue bring-up.
Q_RATE = (115.0, 115.0, 105.0)  # sync, scalar, gpsimd
Q_LAG = (0.0, 0.5, 2.0)

F32 = mybir.dt.float32
F16 = mybir.dt.float16
U32 = mybir.dt.uint32


def _piece_schedule(n_chunk):
    """Static greedy earliest-finish assignment of x pieces to queues.

    Returns {chunk: [(tensor, half, quarter|None, queue_idx), ...]}.
    tensor: 0=xh 1=xl.  Pieces are 512KB halves; first/last two chunks
    are split into 256KB quarters for ramp and finish balance.
    """
    fin = list(Q_LAG)
    sched = {c: [] for c in range(n_chunk)}
    for c in range(n_chunk):
        quartered = c < 2 or c >= n_chunk - 2
        for t in range(2):
            for h in range(2):
                for qtr in (0, 1) if quartered else (None,):
                    nbytes = 128 * HKCH * CHUNK * 2 // (2 if quartered else 1)
                    cost = [nbytes / (r * 1e3) for r in Q_RATE]  # us
                    q = min(range(3), key=lambda i: fin[i] + cost[i])
                    fin[q] += cost[q]
                    sched[c].append((t, h, qtr, q))
    return sched


def build_program(tok_per_core=TOK_PER_CORE):
    n_chunk = tok_per_core // CHUNK
    nc = bacc.Bacc(
        "TRN2", target_bir_lowering=False, debug=False, num_devices=N_CORES
    )
    xh_d = nc.dram_tensor(
        "xh", [n_chunk, 2, 128, HKCH * CHUNK], F16, kind="ExternalInput"
    ).ap()
    xl_d = nc.dram_tensor(
        "xl", [n_chunk, 2, 128, HKCH * CHUNK], F16, kind="ExternalInput"
    ).ap()
    w_d = nc.dram_tensor("w", [D, E], F32, kind="ExternalInput").ap()
    scores_d = nc.dram_tensor(
        "scores", [128, n_chunk, SUBS, TOPK], F32, kind="ExternalOutput"
    ).ap()
    experts_d = nc.dram_tensor(
        "experts", [128, n_chunk, SUBS, TOPK], U32, kind="ExternalOutput"
    ).ap()

    with tile.TileContext(nc) as tc, ExitStack() as ctx:
        wpool = ctx.enter_context(tc.tile_pool(name="wpool", bufs=1))
        xpool = ctx.enter_context(tc.tile_pool(name="xpool", bufs=PIPE))
        ltpool = ctx.enter_context(tc.tile_pool(name="ltpool", bufs=3))
        ptpool = ctx.enter_context(tc.tile_pool(name="ptpool", bufs=3, space="PSUM"))
        ppool = ctx.enter_context(tc.tile_pool(name="ppool", bufs=4, space="PSUM"))
        spool = ctx.enter_context(tc.tile_pool(name="spool", bufs=4))

        ident = wpool.tile([128, 128], F32)
        masks.make_identity(nc, ident[:])

        # w split on DEVICE with the same DVE fp32->fp16 conversions as
        # the proven kernel; packed side by side: whl = [wh | wl]
        w_t = wpool.tile([128, KCH, E], F32)
        nc.gpsimd.dma_start(
            out=w_t[:], in_=w_d.rearrange("(k p) e -> p k e", p=128)
        )
        whl = wpool.tile([128, KCH, 2 * E], F16)
        nc.vector.tensor_copy(whl[:, :, 0:E], w_t[:])
        wl32 = wpool.tile([128, KCH, E], F32)
        nc.vector.tensor_sub(wl32[:], w_t[:], whl[:, :, 0:E])
        nc.vector.tensor_copy(whl[:, :, E : 2 * E], wl32[:])

        # resident output accumulators, flushed in batched DMAs at the end
        sc_all = wpool.tile([128, n_chunk, SUBS, TOPK], F32)
        idx_all = wpool.tile([128, n_chunk, SUBS, TOPK], U32)

        qengines = [nc.sync, nc.scalar, nc.gpsimd]
        sched = _piece_schedule(n_chunk)

        xtiles = {}

        def issue_loads(c):
            xh_t = xpool.tile([128, KCH, CHUNK], F16, tag="xh_t")
            xl_t = xpool.tile([128, KCH, CHUNK], F16, tag="xl_t")
            xtiles[c] = (xh_t, xl_t)
            Q = HKCH * CHUNK // 2
            for t, h, qtr, q in sched[c]:
                dst_t = xh_t if t == 0 else xl_t
                src = (xh_d if t == 0 else xl_d)[c, h]
                dst = dst_t[:, h * HKCH : (h + 1) * HKCH, :]
                if qtr is None:
                    qengines[q].dma_start(out=dst, in_=src)
                elif qtr == 0:
                    qengines[q].dma_start(
                        out=dst[:, 0 : HKCH // 2, :], in_=src[:, 0:Q]
                    )
                else:
                    qengines[q].dma_start(
                        out=dst[:, HKCH // 2 : HKCH, :], in_=src[:, Q:]
                    )

        def compute_range(c, xh_t, xl_t, t0, nt, lps):
            """Full gate chain for tokens [t0, t0+nt) of chunk c.

            lps is a PSUM tile [128, nt]; nt must be a multiple of 128.
            """
            ns = nt // 128
            s0 = t0 // 128
            # matmul chain; partitions 0-63 see xh@wh k0..7 then xl@wh
            # k0..7 — bit-identical to the proven kernel's partial order
            for k in range(KCH):
                nc.tensor.matmul(
                    lps[:], whl[:, k, :], xh_t[:, k, t0 : t0 + nt],
                    start=(k == 0), stop=False,
                )
            for k in range(KCH):
                nc.tensor.matmul(
                    lps[:], whl[:, k, :], xl_t[:, k, t0 : t0 + nt],
                    start=False, stop=(k == KCH - 1),
                )

            # PSUM -> SBUF on the scalar engine (frees DVE)
            ltile = ltpool.tile([128, CHUNK], F32, tag="ltile", name="ltile")
            nc.scalar.activation(
                ltile[:, 0:nt], lps[:], mybir.ActivationFunctionType.Copy
            )

            # transposes into one PSUM bank -> [tok, s, B-col | W-col]
            tpb = ppool.tile([128, SUBS, 128], F32, tag="tpb", name="tpb")
            for s in range(ns):
                nc.tensor.transpose(
                    tpb[:, s, :], ltile[:, s * 128 : (s + 1) * 128], ident[:]
                )
            tp = tpb[:, 0:ns]
            # stage the W half through SBUF (ops may read only one PSUM
            # operand, and gpsimd none)
            wlp4 = spool.tile([128, SUBS, E], F32, tag="wlp4", name="wlp4")
            nc.scalar.activation(
                wlp4[:, 0:ns], tp[:, :, E : 2 * E],
                mybir.ActivationFunctionType.Copy,
            )
            wl = wlp4[:, 0:ns]

            # Dekker TwoSum: s4 + er4 == B + W exactly
            s4 = spool.tile([128, SUBS, E], F32, tag="s4", name="s4")
            nc.vector.tensor_add(s4[:, 0:ns], tp[:, :, 0:E], wl)
            bb4 = spool.tile([128, SUBS, E], F32, tag="bb4", name="bb4")
            nc.vector.tensor_sub(bb4[:, 0:ns], s4[:, 0:ns], tp[:, :, 0:E])
            er4 = spool.tile([128, SUBS, E], F32, tag="er4", name="er4")
            nc.vector.tensor_sub(er4[:, 0:ns], wl, bb4[:, 0:ns])
            # refined selection key r = (s - max) + err (exact shift for
            # contenders, then one rounding at ~ulp(10))
            m4 = spool.tile([128, SUBS, 1], F32, tag="m4", name="m4")
            nc.vector.tensor_reduce(
                m4[:, 0:ns, 0], s4[:, 0:ns], mybir.AxisListType.X,
                mybir.AluOpType.max,
            )
            d4 = spool.tile([128, SUBS, E], F32, tag="d4", name="d4")
            nc.vector.tensor_sub(
                d4[:, 0:ns], s4[:, 0:ns],
                m4[:, 0:ns].broadcast_to((128, ns, E)),
            )
            r4 = spool.tile([128, SUBS, E], F32, tag="r4", name="r4")
            nc.vector.tensor_add(r4[:, 0:ns], d4[:, 0:ns], er4[:, 0:ns])

            vals4 = spool.tile([128, SUBS, TOPK], F32, tag="vals4", name="vals4")
            for s in range(ns):
                nc.vector.max(vals4[:, s, :], r4[:, s, :])
                nc.vector.max_index(
                    idx_all[:, c, s0 + s, :], vals4[:, s, :], r4[:, s, :]
                )

            # batched softmax over the sub-tiles: exp(v - max) / sum
            # (r is the logit shifted per token, so scores are unchanged)
            sh4 = spool.tile([128, SUBS, TOPK], F32, tag="sh4", name="sh4")
            nc.vector.tensor_sub(
                sh4[:, 0:ns],
                vals4[:, 0:ns],
                vals4[:, 0:ns, 0:1].broadcast_to((128, ns, TOPK)),
            )
            ex4 = spool.tile([128, SUBS, TOPK], F32, tag="ex4", name="ex4")
            nc.scalar.activation(
                ex4[:, 0:ns], sh4[:, 0:ns], mybir.ActivationFunctionType.Exp
            )
            sums4 = spool.tile([128, SUBS, 1], F32, tag="sums4", name="sums4")
            nc.vector.tensor_reduce(
                sums4[:, 0:ns, 0],
                ex4[:, 0:ns],
                mybir.AxisListType.X,
                mybir.AluOpType.add,
            )
            rs4 = spool.tile([128, SUBS, 1], F32, tag="rs4", name="rs4")
            nc.vector.reciprocal(rs4[:, 0:ns], sums4[:, 0:ns])
            nc.vector.tensor_mul(
                sc_all[:, c, s0 : s0 + ns],
                ex4[:, 0:ns],
                rs4[:, 0:ns].broadcast_to((128, ns, TOPK)),
            )

        def compute(c):
            xh_t, xl_t = xtiles.pop(c)
            if c == n_chunk - 1:
                # last chunk in two 256-token halves: the post-load serial
                # chain (which lands in a throttled window) is halved
                for m in range(2):
                    lps = ptpool.tile(
                        [128, CHUNK // 2], F32, tag="lps", name="lps",
                        padded_shape=[128, CHUNK],
                    )
                    compute_range(c, xh_t, xl_t, m * 256, 256, lps)
            else:
                lps = ptpool.tile([128, CHUNK], F32, tag="lps", name="lps")
                compute_range(c, xh_t, xl_t, 0, CHUNK, lps)

        for step in range(n_chunk + DELTA):
            if step < n_chunk:
                issue_loads(step)
            if step >= DELTA:
                compute(step - DELTA)

        # batched output flush on sync: bulk during the drain window,
        # last two chunks as a short serial tail
        cut = n_chunk - 2
        nc.sync.dma_start(out=scores_d[:, 0:cut], in_=sc_all[:, 0:cut])
        nc.sync.dma_start(out=experts_d[:, 0:cut], in_=idx_all[:, 0:cut])
        nc.sync.dma_start(out=scores_d[:, cut:], in_=sc_all[:, cut:])
        nc.sync.dma_start(out=experts_d[:, cut:], in_=idx_all[:, cut:])

    nc.compile()
    return nc


_PROGRAM = None


def _get_program():
    global _PROGRAM
    if _PROGRAM is None:
        _PROGRAM = build_program()
    return _PROGRAM


def _make_in_maps(x, weights):
    x = np.asarray(x, dtype=np.float32)
    w = np.asarray(weights, dtype=np.float32)
    maps = []
    for i in range(N_CORES):
        xs = np.ascontiguousarray(x[i * TOK_PER_CORE : (i + 1) * TOK_PER_CORE].T)
        xh = xs.astype(np.float16)
        xl = (xs - xh.astype(np.float32)).astype(np.float16)

        def pack(a):
            # [1024, 8192] -> [chunk, half, 128p, 4k, 512t] contiguous
            return np.ascontiguousarray(
                a.reshape(2, HKCH, 128, N_CHUNK, CHUNK).transpose(3, 0, 2, 1, 4)
            ).reshape(N_CHUNK, 2, 128, HKCH * CHUNK)

        maps.append({"xh": pack(xh), "xl": pack(xl), "w": w})
    return maps


def run(x, weights, trace=False):
    nc = _get_program()
    res = run_bass_kernel_spmd(
        nc, _make_in_maps(x, weights), list(range(N_CORES)), trace=trace
    )

    def unblock(a):
        # [128, n_chunk, SUBS, TOPK] -> [tok_per_core, TOPK]
        # token t = c*CHUNK + s*128 + p
        return np.ascontiguousarray(a.transpose(1, 2, 0, 3)).reshape(-1, TOPK)

    scores = np.concatenate(
        [unblock(res.results[i]["scores"]) for i in range(N_CORES)]
    )
    experts = np.concatenate(
        [unblock(res.results[i]["experts"]).astype(np.int32) for i in range(N_CORES)]
    )
    return (scores, experts), res


def kernel(x, weights):
    out, _ = run(x, weights)
    return out
